# revision 34
# baseline (speedup 1.0000x reference)
"""Multi-head self-attention (RoPE, causal) Bass kernel for 8 TRN2 NeuronCores.

Problem: x (2, 2048, 1024) f32, wqkv (3072, 1024), wo (1024, 1024).
  qkv = x @ wqkv.T ; RoPE(q, k) ; causal softmax attention (16 heads, hd=64);
  out = y @ wo.T.

Sharding: batch (2-way) x head-group (4-way) tensor parallel = 8 cores.
Each core computes a full (2048, 1024) partial output for its batch from its
4 heads; host sums the 4 partials per batch (the TP all-reduce done at
unshard time).

Schedule: single fused instruction stream per engine.  QKV chunk j+1 and
the wo projection of chunk j-1 are spliced between attention tiles of
chunk j so the PE never idles (idle resets the PE clock to mid p-state).
Scores+exp run a few tiles ahead of the P@V consumers (pt ring) so the
Act engine streams continuously.  DMA triggers round-robin over 4 engine
queues (a single sequencer serializes triggers at ~0.6us each), and big
tensors are split across DMA queues (one queue sustains only ~22 GB/s).
"""
import sys

sys.path.insert(0, "/opt/trn_rl_repo")

import numpy as np

import concourse.bass as bass
import concourse.mybir as mybir
import concourse.tile as tile
from concourse import bacc, bass_utils
from concourse.masks import make_identity

B, L, D = 2, 2048, 1024
NH, HD = 16, 64
NCORES = 8
HPC = 4            # heads per core
LQB = 512          # Lq block per chunk
NLQ = L // LQB     # 4
NLT = L // 128     # 16
KT = D // 128      # 8 contraction tiles for projections

F32 = mybir.dt.float32
F32R = mybir.dt.float32r
F16 = mybir.dt.float16
BF16 = mybir.dt.bfloat16

_cache = {}


DEFAULT_OPTS = {
    "look": 3,          # se units emitted ahead of pv units
    "pe_mask": True,    # diag causal mask on PE (negi/bigm) vs DVE tri-mul
    "warms": 4,         # standalone warmup matmuls
    "r2_gpsimd": False, # round-2 DMA triggers on gpsimd only
    "halves": 4,        # round-1 k-tiles split into partition halves
}


def build_nc(debug=False, **opts):
    o = dict(DEFAULT_OPTS)
    o.update(opts)
    nc = bacc.Bacc("TRN2", target_bir_lowering=False, debug=False)

    XT = nc.dram_tensor("XT", [D, L], F16, kind="ExternalInput")
    WQKT = nc.dram_tensor("WQKT", [D, 512], F16, kind="ExternalInput")
    WVT = nc.dram_tensor("WVT", [D, 260], F16, kind="ExternalInput")
    WOT = nc.dram_tensor("WOT", [HPC * HD, D], F16, kind="ExternalInput")
    PERM = nc.dram_tensor("PERM", [128, 128], F32R, kind="ExternalInput")
    COS = nc.dram_tensor("COS", [128, L], F16, kind="ExternalInput")
    SIN = nc.dram_tensor("SIN", [128, L], F16, kind="ExternalInput")
    OUT = nc.dram_tensor("OUT", [L, D], F16, kind="ExternalOutput")

    # round-robin DMA trigger issue over 4 sequencers: one sequencer
    # serializes triggers at ~0.6us each
    trig = {"i": 0}

    with tile.TileContext(nc) as tc:
        with (
            tc.tile_pool(name="consts", bufs=1) as cpool,
            tc.tile_pool(name="weights", bufs=1) as wpool,
            tc.tile_pool(name="qkrot", bufs=1) as rotpool,
            tc.tile_pool(name="vsb", bufs=1) as vpool,
            tc.tile_pool(name="yall", bufs=1) as ypool,
            tc.tile_pool(name="ytr", bufs=1) as ytpool,
            tc.tile_pool(name="xt", bufs=16) as xpool,
            tc.tile_pool(name="tmps", bufs=2) as tpool,
            tc.tile_pool(name="raws", bufs=3) as rawpool,
            tc.tile_pool(name="pts", bufs=6) as ptpool,
            tc.tile_pool(name="outsb", bufs=3) as opool,
            tc.tile_pool(name="recs", bufs=4) as recpool,
            tc.tile_pool(name="psS", bufs=2, space="PSUM") as sppool,
            tc.tile_pool(name="psQ", bufs=2, space="PSUM") as qpool,
            tc.tile_pool(name="psY", bufs=1, space="PSUM") as ypspool,
        ):
            engines = [nc.sync, nc.scalar, nc.gpsimd]

            def dma_in(dst, src):
                eng = engines[trig["i"] % 3]
                trig["i"] += 1
                eng.dma_start(dst, src)

            # ---- on-chip constants first (no DMA dependency) -----------
            idn = cpool.tile([128, 128], F16, tag="idn", name="idn")
            make_identity(nc, idn[:])
            tri = cpool.tile([128, 128], F16, tag="tri", name="tri")
            # tri[k, q] = 1 where q >= k else 0 (causal keep-mask for the
            # diagonal 128x128 block of S^T, applied to exp(S) on DVE)
            nc.gpsimd.memset(tri[:], 1.0)
            nc.gpsimd.affine_select(
                out=tri[:], in_=tri[:],
                compare_op=mybir.AluOpType.is_ge, fill=0.0, base=0,
                pattern=[[1, 128]], channel_multiplier=-1,
            )
            if o["pe_mask"]:
                negi_sb = cpool.tile([128, 128], BF16, tag="negi",
                                     name="negi_sb")
                nc.gpsimd.memset(negi_sb[:], -1e9)
                nc.gpsimd.affine_select(
                    out=negi_sb[:], in_=negi_sb[:],
                    compare_op=mybir.AluOpType.is_equal, fill=0.0, base=0,
                    pattern=[[-1, 128]], channel_multiplier=1,
                )
                bigm_sb = cpool.tile([128, 128], BF16, tag="bigm",
                                     name="bigm_sb")
                nc.gpsimd.memset(bigm_sb[:], 1.0)
                nc.gpsimd.affine_select(
                    out=bigm_sb[:], in_=bigm_sb[:],
                    compare_op=mybir.AluOpType.is_gt, fill=0.0, base=0,
                    pattern=[[-1, 128]], channel_multiplier=1,
                )
            wtile = cpool.tile([128, 512], F16, tag="warm", name="wtile")
            nc.vector.memset(wtile[:], 0.125)

            # ---- DMA triggers, priority order ---------------------------
            # round 1: wqk + xt chunk 0; k<4 split into 64-partition halves
            # so the first k-tiles land early
            wqk_sb = [wpool.tile([128, 512], F16, tag=f"wqk{k}", name=f"wqk{k}")
                      for k in range(KT)]
            xt_t = {}
            for j in range(NLQ):
                for k in range(KT):
                    xt_t[(j, k)] = xpool.tile([128, LQB], F16, tag="xt",
                                              name=f"xt{j}_{k}")
            for k in range(KT):
                xs = slice(0, LQB)
                if k < o["halves"]:
                    for p in range(2):
                        rs = slice(k * 128 + 64 * p, k * 128 + 64 * p + 64)
                        ds = slice(64 * p, 64 * p + 64)
                        dma_in(wqk_sb[k][ds, :], WQKT[rs, :])
                        dma_in(xt_t[(0, k)][ds, :], XT[rs, xs])
                else:
                    rs = slice(k * 128, (k + 1) * 128)
                    dma_in(wqk_sb[k][:], WQKT[rs, :])
                    dma_in(xt_t[(0, k)][:], XT[rs, xs])
            # round 2: rope tables (per-chunk splits), perm, wv
            def dma_r2(dst, srcap):
                if o["r2_gpsimd"]:
                    nc.gpsimd.dma_start(dst, srcap)
                else:
                    dma_in(dst, srcap)

            cos_sb = cpool.tile([128, L], F16, tag="cos", name="cos_sb")
            sin_sb = cpool.tile([128, L], F16, tag="sin", name="sin_sb")
            for j in range(NLQ):
                xs = slice(j * LQB, (j + 1) * LQB)
                dma_r2(cos_sb[:, xs], COS[:, xs])
                dma_r2(sin_sb[:, xs], SIN[:, xs])
            perm_sb = cpool.tile([128, 128], F32R, tag="perm", name="perm_sb")
            dma_r2(perm_sb[:], PERM[:, :])
            wvt_sb = []
            for k in range(KT):
                wv = wpool.tile([128, 260], F16, tag=f"wv{k}", name=f"wv{k}")
                dma_r2(wv[:], WVT[k * 128:(k + 1) * 128, :])
                wvt_sb.append(wv)
            # rounds 3+: xt chunks 1-3, wo weights — sync queue only (these
            # have slack; keep scalar/gpsimd queues clear for compute)
            for k in range(KT):
                nc.sync.dma_start(xt_t[(1, k)][:],
                                  XT[k * 128:(k + 1) * 128, LQB:2 * LQB])
            wot_sb = []
            for c2 in range(2):
                w = wpool.tile([128, D], F16, tag=f"wo{c2}", name=f"wo{c2}")
                for p in range(2):
                    nc.sync.dma_start(
                        w[64 * p:64 * p + 64, :],
                        WOT[c2 * 128 + 64 * p:c2 * 128 + 64 * p + 64, :])
                wot_sb.append(w)
            for j in (2, 3):
                xs = slice(j * LQB, (j + 1) * LQB)
                for k in range(KT):
                    nc.sync.dma_start(xt_t[(j, k)][:],
                                      XT[k * 128:(k + 1) * 128, xs])

            # persistent activation storage
            qk_rot = [rotpool.tile([128, L], F32R, tag=f"rot{m}", name=f"rot{m}")
                      for m in range(4)]
            v_sb = [vpool.tile([128, 260], F16, tag=f"v{t}", name=f"v{t}")
                    for t in range(NLT)]
            y_all = [ypool.tile([128, HPC * HD], F16, tag=f"y{i}", name=f"y{i}")
                     for i in range(NLT)]
            yt_sb = [ytpool.tile([128, L], F16, tag=f"yt{c2}", name=f"yt{c2}")
                     for c2 in range(2)]
            y_ps = [ypspool.tile([128, 260], F32, tag=f"yps{h}", name=f"yps{h}")
                    for h in range(2)]

            # ---- PE warmup: ramp the clock while round-1 DMA lands ------
            def warm_mm():
                ws = sppool.tile([128, 1024], F32, tag="sp", name="warm")
                nc.tensor.matmul(ws[:, 0:512], idn[:], wtile[:],
                                 start=True, stop=True)

            for _ in range(o["warms"]):
                warm_mm()

            # ---- work units --------------------------------------------
            def qkv_chain(j, m, interleave_with=None):
                """8 k-tile matmuls for q/k head-pair m of chunk j -> psum.
                m: 0,1 -> q pairs (h01, h23); 2,3 -> k pairs."""
                ps = qpool.tile([128, 512], F32, tag="q", name=f"ps{j}_{m}")
                for k in range(KT):
                    nc.tensor.matmul(
                        ps[:], wqk_sb[k][:, m * 128:(m + 1) * 128],
                        xt_t[(j, k)][:],
                        start=(k == 0), stop=(k == KT - 1),
                    )
                return ps

            def qkv_chain2(j, ma, mb, warm=False):
                """Two m-chains interleaved k-by-k (for the DMA-paced j=0)."""
                pa = qpool.tile([128, 512], F32, tag="q", name=f"ps{j}_{ma}")
                pb = qpool.tile([128, 512], F32, tag="q", name=f"ps{j}_{mb}")
                for k in range(KT):
                    nc.tensor.matmul(
                        pa[:], wqk_sb[k][:, ma * 128:(ma + 1) * 128],
                        xt_t[(j, k)][:],
                        start=(k == 0), stop=(k == KT - 1),
                    )
                    nc.tensor.matmul(
                        pb[:], wqk_sb[k][:, mb * 128:(mb + 1) * 128],
                        xt_t[(j, k)][:],
                        start=(k == 0), stop=(k == KT - 1),
                    )
                    if warm and k < 7:
                        warm_mm()
                return pa, pb

            def qkv_post(j, m, ps):
                """rope for head-pair m of chunk j: copy psum->sbuf f32r,
                perm matmul, then q*cos + perm(q)*sin on DVE."""
                xs = slice(j * LQB, (j + 1) * LQB)
                raw = rawpool.tile([128, LQB], F32R, tag="raw", name="raw")
                nc.vector.tensor_copy(raw[:], ps[:])
                psw = qpool.tile([128, 512], F32, tag="q", name=f"psw{j}_{m}")
                nc.tensor.matmul(psw[:], perm_sb[:], raw[:],
                                 start=True, stop=True)
                t1 = tpool.tile([128, LQB], F32, tag="t1", name="t1")
                nc.vector.tensor_mul(t1[:], raw[:].bitcast(F32), cos_sb[:, xs])
                t2 = tpool.tile([128, LQB], F32, tag="t2", name="t2")
                nc.vector.tensor_mul(t2[:], psw[:], sin_sb[:, xs])
                nc.vector.tensor_add(qk_rot[m][:, xs], t1[:], t2[:])

            def v_unit(j, i2):
                """one L-tile of V for chunk j: matmul chain + copy."""
                ti = j * 4 + i2
                psv = qpool.tile([128, 512], F32, tag="q", name=f"psv{ti}")
                for k in range(KT):
                    nc.tensor.matmul(
                        psv[:, 0:260], xt_t[(j, k)][:, i2 * 128:(i2 + 1) * 128],
                        wvt_sb[k][:],
                        start=(k == 0), stop=(k == KT - 1),
                    )
                nc.scalar.copy(v_sb[ti][:], psv[:, 0:260])
                nc.gpsimd.memset(v_sb[ti][:, 64:260:65], 1.0)

            # attention tile units: se = scores+exp(+mask), pv = P@V
            pt_of = {}

            def att_se(jq, hp, t):
                nt = 4 * jq + 4
                diag = t >= 4 * jq
                off = max(0, t * 128 - jq * LQB)
                off_mm = min(off, 256)   # f32r needs >=256 moving rows
                r = off // 128
                ks = slice(t * 128, (t + 1) * 128)
                sp = sppool.tile([128, 1024], F32, tag="sp", name="sp")
                for h in range(2):
                    hs = slice(64 * h, 64 * h + 64)
                    nc.tensor.matmul(
                        sp[:, 512 * h + off_mm:512 * h + 512],
                        qk_rot[2 + hp][hs, ks],
                        qk_rot[hp][hs, jq * LQB + off_mm:(jq + 1) * LQB],
                        start=True, stop=not (diag and o["pe_mask"]),
                    )
                if diag and o["pe_mask"]:
                    for h in range(2):
                        nc.tensor.matmul(
                            sp[:, 512 * h + off:512 * h + off + 128],
                            negi_sb[:], bigm_sb[:],
                            start=False, stop=True,
                        )
                pt = ptpool.tile([128, 1024], F16, tag="pt", name="pt")
                if off >= 256:
                    for h in range(2):
                        nc.scalar.activation(
                            pt[:, 512 * h + off:512 * h + 512],
                            sp[:, 512 * h + off:512 * h + 512],
                            mybir.ActivationFunctionType.Exp)
                else:
                    nc.scalar.activation(
                        pt[:, off:1024], sp[:, off:1024],
                        mybir.ActivationFunctionType.Exp)
                if diag and not o["pe_mask"]:
                    # zero exp(S) above the diagonal in the diag 128-block
                    for h in range(2):
                        c0 = 512 * h + 128 * r
                        nc.vector.tensor_mul(pt[:, c0:c0 + 128],
                                             pt[:, c0:c0 + 128], tri[:])
                pt_of[(jq, hp, t)] = pt

            def att_pv(jq, hp, t):
                nt = 4 * jq + 4
                off = max(0, t * 128 - jq * LQB)
                r = off // 128
                if t == 0:
                    # zero via DVE, then accumulate with start=False
                    # throughout: matmul start=True zeroes the WHOLE psum
                    # bank, which clobbers sibling js-regions when Tile
                    # reorders the (commutative) accumulate matmuls.  Emitted
                    # here (not in att_se) so the DVE stream keeps it after
                    # the previous pair's normalize.
                    for h in range(2):
                        nc.vector.memset(y_ps[h][:], 0.0)
                pt = pt_of.pop((jq, hp, t))
                for h in range(2):
                    H = 2 * hp + h
                    for js in range(r, 4):
                        nc.tensor.matmul(
                            y_ps[h][:, 65 * js:65 * js + 65],
                            pt[:, 512 * h + 128 * js:512 * h + 128 * js + 128],
                            v_sb[t][:, 65 * H:65 * H + 65],
                            start=False, stop=(t == nt - 1),
                            skip_group_check=True,
                        )
                if t == nt - 1:
                    for h in range(2):
                        H = 2 * hp + h
                        rec = recpool.tile([128, 4], F32, tag="rec", name="rec")
                        nc.vector.reciprocal(rec[:], y_ps[h][:, 64:260:65])
                        for js in range(4):
                            i = 4 * jq + js
                            nc.vector.tensor_scalar_mul(
                                y_all[i][:, HD * H:HD * H + HD],
                                y_ps[h][:, 65 * js:65 * js + 64],
                                rec[:, js:js + 1],
                            )

            def wo_unit(i, fine=False):
                """project one finished L-tile through wo and stream out.
                fine=True (tail tiles): split copies over both engines and
                the OUT DMA over 4 queues/triggers to shrink the tail."""
                tp = qpool.tile([128, 512], F32, tag="q", name=f"tp{i}")
                tp16 = tp[:].bitcast(F16)
                for c2 in range(2):
                    nc.tensor.transpose(
                        tp16[:, 128 * c2:128 * c2 + 128],
                        y_all[i][:, 128 * c2:128 * c2 + 128],
                        idn[:],
                    )
                    nc.vector.tensor_copy(
                        yt_sb[c2][:, 128 * i:128 * i + 128],
                        tp16[:, 128 * c2:128 * c2 + 128],
                    )
                ob = opool.tile([128, 1024], F16, tag="ob", name="ob")
                out_engs = [nc.gpsimd, nc.scalar, nc.sync, nc.gpsimd]
                for half in range(2):
                    po = qpool.tile([128, 512], F32, tag="q", name=f"po{i}")
                    for c2 in range(2):
                        nc.tensor.matmul(
                            po[:],
                            yt_sb[c2][:, 128 * i:128 * i + 128],
                            wot_sb[c2][:, 512 * half:512 * half + 512],
                            start=(c2 == 0), stop=(c2 == 1),
                        )
                    hs = slice(512 * half, 512 * half + 512)
                    if fine:
                        for quad in range(2):
                            qs = slice(512 * half + 256 * quad,
                                       512 * half + 256 * quad + 256)
                            ps = slice(256 * quad, 256 * quad + 256)
                            if quad == 0:
                                nc.scalar.copy(ob[:, qs], po[:, ps])
                            else:
                                nc.vector.tensor_copy(ob[:, qs], po[:, ps])
                            out_engs[2 * half + quad].dma_start(
                                OUT[128 * i:128 * i + 128, qs], ob[:, qs])
                    else:
                        if half == 0:
                            nc.scalar.copy(ob[:, hs], po[:])
                        else:
                            nc.vector.tensor_copy(ob[:, hs], po[:])
                        nc.gpsimd.dma_start(OUT[128 * i:128 * i + 128, hs],
                                            ob[:, hs])

            # ---- emission: fused global schedule ------------------------
            # qkv chunk 0 (DMA-paced: interleave the two hp0 chains k-by-k
            # and sprinkle warmup matmuls)
            pa, pb = qkv_chain2(0, 0, 2, warm=True)
            qkv_post(0, 0, pa)
            qkv_post(0, 2, pb)
            pa, pb = qkv_chain2(0, 1, 3)
            qkv_post(0, 1, pa)
            qkv_post(0, 3, pb)

            for jq in range(NLQ):
                nt = 4 * jq + 4
                # fill units: qkv of chunk jq+1, v of chunk jq (needed by
                # this chunk's diag tiles) emitted first, wo of chunk jq-1
                fills = []
                for i2 in range(4):
                    fills.append(lambda j=jq, i2=i2: v_unit(j, i2))
                if jq + 1 < NLQ:
                    def mk_chain2(ma, mb):
                        def f(j=jq + 1, ma=ma, mb=mb):
                            pa, pb = qkv_chain2(j, ma, mb)
                            qkv_post(j, ma, pa)
                            qkv_post(j, mb, pb)
                        return f
                    fills.append(mk_chain2(0, 2))
                    fills.append(mk_chain2(1, 3))
                if jq > 0:
                    for i in range(4 * (jq - 1), 4 * (jq - 1) + 4):
                        fills.append(lambda i=i: wo_unit(i))

                units = [(hp, t) for hp in range(2) for t in range(nt)]
                LOOK = o["look"]
                nf = len(fills)
                nu = len(units)
                fi = 0

                def need_fills(hp, t):
                    # v units (fills[0..3]) must be emitted before the pv
                    # (and se, via lookahead) units that consume v_sb of
                    # this chunk's diagonal tiles: a consumer emitted before
                    # its producer reads stale SBUF (Tile orders by
                    # emission, it does not reorder to satisfy RAW).
                    d = t - 4 * jq
                    return min(4, d + 1) if d >= 0 else 0

                for ui, (hp, t) in enumerate(units):
                    if ui == 0:
                        for la in range(min(LOOK, nu)):
                            att_se(jq, *units[la])
                    elif ui + LOOK - 1 < nu:
                        att_se(jq, *units[ui + LOOK - 1])
                    # spread fills across the pv walk
                    while fi < max(nf * (ui + 1) // nu, need_fills(hp, t)):
                        fills[fi]()
                        fi += 1
                    att_pv(jq, hp, t)
                while fi < nf:
                    fills[fi]()
                    fi += 1

            for i in range(4 * (NLQ - 1), 4 * NLQ):
                wo_unit(i, fine=(i >= 4 * NLQ - 2))

    nc.finalize()
    return nc


def prep_inputs(x, wqkv, wo):
    """Build the 8 per-core input dicts from the full-problem inputs."""
    x = np.asarray(x, dtype=np.float32)
    wqkv = np.asarray(wqkv, dtype=np.float32)
    wo = np.asarray(wo, dtype=np.float32)

    # rope tables
    inv_freq = 1.0 / (10000.0 ** (np.arange(0, HD, 2, dtype=np.float32) / HD))
    t = np.arange(L, dtype=np.float32)
    freqs = np.outer(t, inv_freq)                  # (L, 32)
    cos32 = np.cos(freqs).T.astype(np.float32)     # (32, L)
    sin32 = np.sin(freqs).T.astype(np.float32)
    COS = np.ascontiguousarray(np.tile(cos32, (4, 1)))           # (128, L)
    SIN = np.ascontiguousarray(
        np.concatenate([-sin32, sin32, -sin32, sin32], axis=0)
    )

    # 32-block swap permutation (within each head's 64 rows)
    PERM = np.zeros((128, 128), dtype=np.float32)
    for blk in range(2):
        o = 64 * blk
        PERM[o:o + 32, o + 32:o + 64] = np.eye(32)
        PERM[o + 32:o + 64, o:o + 32] = np.eye(32)

    in_maps = []
    scale = np.float32(HD ** -0.5)
    for c in range(NCORES):
        b, g = divmod(c, 4)
        qrows = slice(256 * g, 256 * g + 256)
        krows = slice(1024 + 256 * g, 1024 + 256 * g + 256)
        vrows = slice(2048 + 256 * g, 2048 + 256 * g + 256)

        XT = np.ascontiguousarray(x[b].T)                        # (1024, 2048)
        wq = (wqkv[qrows, :] * scale).T                          # (1024, 256)
        wk = wqkv[krows, :].T
        WQKT = np.ascontiguousarray(np.concatenate([wq, wk], axis=1))
        vpart = wqkv[vrows, :].T                                 # (1024, 256)
        WVT = np.zeros((D, 260), dtype=np.float32)
        for h in range(HPC):
            WVT[:, 65 * h:65 * h + 64] = vpart[:, 64 * h:64 * h + 64]
        WOT = np.ascontiguousarray(wo[:, 256 * g:256 * g + 256].T)

        in_maps.append({
            "XT": XT.astype(np.float16),
            "WQKT": WQKT.astype(np.float16),
            "WVT": WVT.astype(np.float16),
            "WOT": WOT.astype(np.float16),
            "COS": COS.astype(np.float16),
            "SIN": SIN.astype(np.float16),
            "PERM": PERM,
        })
    return in_maps


def kernel(x, wqkv, wo):
    if "nc" not in _cache:
        _cache["nc"] = build_nc()
    nc = _cache["nc"]
    in_maps = prep_inputs(x, wqkv, wo)
    res = bass_utils.run_bass_kernel_spmd(nc, in_maps, list(range(NCORES)))
    outs = [res.results[c]["OUT"].astype(np.float32) for c in range(NCORES)]
    out0 = outs[0] + outs[1] + outs[2] + outs[3]
    out1 = outs[4] + outs[5] + outs[6] + outs[7]
    return np.stack([out0, out1]).astype(np.float32)


# revision 37
# speedup vs baseline: 1.2339x; 1.2339x over previous
"""Multi-head self-attention (RoPE, causal) Bass kernel for 8 TRN2 NeuronCores.

Problem: x (2, 2048, 1024) f32, wqkv (3072, 1024), wo (1024, 1024).
  qkv = x @ wqkv.T ; RoPE(q, k) ; causal softmax attention (16 heads, hd=64);
  out = y @ wo.T.

Sharding: batch (2-way) x head-group (4-way) tensor parallel = 8 cores.
Each core computes a full (2048, 1024) partial output for its batch from its
4 heads; host sums the 4 partials per batch (the TP all-reduce done at
unshard time).

Schedule: single fused instruction stream per engine.  QKV chunk j+1 and
the wo projection of chunk j-1 are spliced between attention tiles of
chunk j so the PE never idles (idle resets the PE clock to mid p-state).
Scores+exp run a few tiles ahead of the P@V consumers (pt ring) so the
Act engine streams continuously.  DMA triggers round-robin over 4 engine
queues (a single sequencer serializes triggers at ~0.6us each), and big
tensors are split across DMA queues (one queue sustains only ~22 GB/s).
"""
import sys

sys.path.insert(0, "/opt/trn_rl_repo")

import numpy as np

import concourse.bass as bass
import concourse.mybir as mybir
import concourse.tile as tile
from concourse import bacc, bass_utils
from concourse.masks import make_identity

B, L, D = 2, 2048, 1024
NH, HD = 16, 64
NCORES = 8
HPC = 4            # heads per core
LQB = 512          # Lq block per chunk
NLQ = L // LQB     # 4
NLT = L // 128     # 16
KT = D // 128      # 8 contraction tiles for projections

F32 = mybir.dt.float32
F32R = mybir.dt.float32r
F16 = mybir.dt.float16
BF16 = mybir.dt.bfloat16

_cache = {}


DEFAULT_OPTS = {
    "look": 3,          # se units emitted ahead of pv units
    "pe_mask": True,    # diag causal mask on PE (negi/bigm) vs DVE tri-mul
    "warms": 4,         # standalone warmup matmuls
    "r2_gpsimd": False, # round-2 DMA triggers on gpsimd only
    "halves": 4,        # round-1 k-tiles split into partition halves
}


def build_nc(debug=False, **opts):
    o = dict(DEFAULT_OPTS)
    o.update(opts)
    nc = bacc.Bacc("TRN2", target_bir_lowering=False, debug=False)

    XT = nc.dram_tensor("XT", [D, L], F16, kind="ExternalInput")
    WQKT = nc.dram_tensor("WQKT", [D, 512], F16, kind="ExternalInput")
    WVT = nc.dram_tensor("WVT", [128, KT * 260], F16, kind="ExternalInput")
    WOT = nc.dram_tensor("WOT", [HPC * HD, D], F16, kind="ExternalInput")
    PERM = nc.dram_tensor("PERM", [128, 128], F32R, kind="ExternalInput")
    COS = nc.dram_tensor("COS", [128, L], F16, kind="ExternalInput")
    SIN = nc.dram_tensor("SIN", [128, L], F16, kind="ExternalInput")
    OUT = nc.dram_tensor("OUT", [L, D], F16, kind="ExternalOutput")

    # round-robin DMA trigger issue over 4 sequencers: one sequencer
    # serializes triggers at ~0.6us each
    trig = {"i": 0}

    with tile.TileContext(nc) as tc:
        with (
            tc.tile_pool(name="consts", bufs=1) as cpool,
            tc.tile_pool(name="weights", bufs=1) as wpool,
            tc.tile_pool(name="qkrot", bufs=1) as rotpool,
            tc.tile_pool(name="vsb", bufs=1) as vpool,
            tc.tile_pool(name="yall", bufs=1) as ypool,
            tc.tile_pool(name="ytr", bufs=1) as ytpool,
            tc.tile_pool(name="xt", bufs=16) as xpool,
            tc.tile_pool(name="tmps", bufs=2) as tpool,
            tc.tile_pool(name="raws", bufs=3) as rawpool,
            tc.tile_pool(name="pts", bufs=6) as ptpool,
            tc.tile_pool(name="outsb", bufs=3) as opool,
            tc.tile_pool(name="recs", bufs=4) as recpool,
            tc.tile_pool(name="psS", bufs=2, space="PSUM") as sppool,
            tc.tile_pool(name="psQ", bufs=2, space="PSUM") as qpool,
            tc.tile_pool(name="psY", bufs=1, space="PSUM") as ypspool,
        ):
            engines = [nc.sync, nc.scalar, nc.gpsimd]

            def dma_in(dst, src):
                eng = engines[trig["i"] % 3]
                trig["i"] += 1
                eng.dma_start(dst, src)

            # ---- on-chip constants first (no DMA dependency) -----------
            idn = cpool.tile([128, 128], F16, tag="idn", name="idn")
            make_identity(nc, idn[:])
            tri = cpool.tile([128, 128], F16, tag="tri", name="tri")
            # tri[k, q] = 1 where q >= k else 0 (causal keep-mask for the
            # diagonal 128x128 block of S^T, applied to exp(S) on DVE)
            nc.gpsimd.memset(tri[:], 1.0)
            nc.gpsimd.affine_select(
                out=tri[:], in_=tri[:],
                compare_op=mybir.AluOpType.is_ge, fill=0.0, base=0,
                pattern=[[1, 128]], channel_multiplier=-1,
            )
            if o["pe_mask"]:
                negi_sb = cpool.tile([128, 128], BF16, tag="negi",
                                     name="negi_sb")
                nc.gpsimd.memset(negi_sb[:], -1e9)
                nc.gpsimd.affine_select(
                    out=negi_sb[:], in_=negi_sb[:],
                    compare_op=mybir.AluOpType.is_equal, fill=0.0, base=0,
                    pattern=[[-1, 128]], channel_multiplier=1,
                )
                bigm_sb = cpool.tile([128, 128], BF16, tag="bigm",
                                     name="bigm_sb")
                nc.gpsimd.memset(bigm_sb[:], 1.0)
                nc.gpsimd.affine_select(
                    out=bigm_sb[:], in_=bigm_sb[:],
                    compare_op=mybir.AluOpType.is_gt, fill=0.0, base=0,
                    pattern=[[-1, 128]], channel_multiplier=1,
                )
            wtile = cpool.tile([128, 512], F16, tag="warm", name="wtile")
            nc.vector.memset(wtile[:], 0.125)

            # ---- DMA triggers, priority order ---------------------------
            # round 1: wqk + xt chunk 0; k<4 split into 64-partition halves
            # so the first k-tiles land early
            wqk_sb = [wpool.tile([128, 512], F16, tag=f"wqk{k}", name=f"wqk{k}")
                      for k in range(KT)]
            xt_t = {}
            for j in range(NLQ):
                for k in range(KT):
                    xt_t[(j, k)] = xpool.tile([128, LQB], F16, tag="xt",
                                              name=f"xt{j}_{k}")
            for k in range(KT):
                xs = slice(0, LQB)
                if k < o["halves"]:
                    for p in range(2):
                        rs = slice(k * 128 + 64 * p, k * 128 + 64 * p + 64)
                        ds = slice(64 * p, 64 * p + 64)
                        dma_in(wqk_sb[k][ds, :], WQKT[rs, :])
                        dma_in(xt_t[(0, k)][ds, :], XT[rs, xs])
                else:
                    rs = slice(k * 128, (k + 1) * 128)
                    dma_in(wqk_sb[k][:], WQKT[rs, :])
                    dma_in(xt_t[(0, k)][:], XT[rs, xs])
            # round 2: rope tables (per-chunk splits), perm, wv
            def dma_r2(dst, srcap):
                if o["r2_gpsimd"]:
                    nc.gpsimd.dma_start(dst, srcap)
                else:
                    dma_in(dst, srcap)

            # few, fat round-2 triggers: only chunk-0 rope tables are
            # urgent; the rest rides one big DMA per tensor
            cos_sb = cpool.tile([128, L], F16, tag="cos", name="cos_sb")
            sin_sb = cpool.tile([128, L], F16, tag="sin", name="sin_sb")
            perm_sb = cpool.tile([128, 128], F32R, tag="perm", name="perm_sb")
            dma_r2(cos_sb[:, 0:LQB], COS[:, 0:LQB])
            dma_r2(sin_sb[:, 0:LQB], SIN[:, 0:LQB])
            dma_r2(perm_sb[:], PERM[:, :])
            wv_all = wpool.tile([128, KT * 260], F16, tag="wv", name="wv_all")
            dma_r2(wv_all[:, 0:4 * 260], WVT[:, 0:4 * 260])
            dma_r2(wv_all[:, 4 * 260:], WVT[:, 4 * 260:])
            dma_r2(cos_sb[:, LQB:], COS[:, LQB:])
            dma_r2(sin_sb[:, LQB:], SIN[:, LQB:])
            wvt_sb = [wv_all[:, k * 260:(k + 1) * 260] for k in range(KT)]
            # rounds 3+: xt chunks 1-3, wo weights — sync queue only (these
            # have slack; keep scalar/gpsimd queues clear for compute)
            for k in range(KT):
                nc.sync.dma_start(xt_t[(1, k)][:],
                                  XT[k * 128:(k + 1) * 128, LQB:2 * LQB])
            wot_sb = []
            for c2 in range(2):
                w = wpool.tile([128, D], F16, tag=f"wo{c2}", name=f"wo{c2}")
                for p in range(2):
                    nc.sync.dma_start(
                        w[64 * p:64 * p + 64, :],
                        WOT[c2 * 128 + 64 * p:c2 * 128 + 64 * p + 64, :])
                wot_sb.append(w)
            for j in (2, 3):
                xs = slice(j * LQB, (j + 1) * LQB)
                for k in range(KT):
                    nc.sync.dma_start(xt_t[(j, k)][:],
                                      XT[k * 128:(k + 1) * 128, xs])

            # persistent activation storage
            qk_rot = [rotpool.tile([128, L], F32R, tag=f"rot{m}", name=f"rot{m}")
                      for m in range(4)]
            v_sb = [vpool.tile([128, 260], F16, tag=f"v{t}", name=f"v{t}")
                    for t in range(NLT)]
            y_all = [ypool.tile([128, HPC * HD], F16, tag=f"y{i}", name=f"y{i}")
                     for i in range(NLT)]
            yt_sb = [ytpool.tile([128, L], F16, tag=f"yt{c2}", name=f"yt{c2}")
                     for c2 in range(2)]
            y_ps = [ypspool.tile([128, 260], F32, tag=f"yps{h}", name=f"yps{h}")
                    for h in range(2)]

            # ---- PE warmup: ramp the clock while round-1 DMA lands ------
            def warm_mm():
                ws = sppool.tile([128, 1024], F32, tag="sp", name="warm")
                nc.tensor.matmul(ws[:, 0:512], idn[:], wtile[:],
                                 start=True, stop=True)

            for _ in range(o["warms"]):
                warm_mm()

            # ---- work units --------------------------------------------
            def qkv_chain(j, m, interleave_with=None):
                """8 k-tile matmuls for q/k head-pair m of chunk j -> psum.
                m: 0,1 -> q pairs (h01, h23); 2,3 -> k pairs."""
                ps = qpool.tile([128, 512], F32, tag="q", name=f"ps{j}_{m}")
                for k in range(KT):
                    nc.tensor.matmul(
                        ps[:], wqk_sb[k][:, m * 128:(m + 1) * 128],
                        xt_t[(j, k)][:],
                        start=(k == 0), stop=(k == KT - 1),
                    )
                return ps

            def qkv_chain2(j, ma, mb, warm=False):
                """Two m-chains interleaved k-by-k (for the DMA-paced j=0)."""
                pa = qpool.tile([128, 512], F32, tag="q", name=f"ps{j}_{ma}")
                pb = qpool.tile([128, 512], F32, tag="q", name=f"ps{j}_{mb}")
                for k in range(KT):
                    nc.tensor.matmul(
                        pa[:], wqk_sb[k][:, ma * 128:(ma + 1) * 128],
                        xt_t[(j, k)][:],
                        start=(k == 0), stop=(k == KT - 1),
                    )
                    nc.tensor.matmul(
                        pb[:], wqk_sb[k][:, mb * 128:(mb + 1) * 128],
                        xt_t[(j, k)][:],
                        start=(k == 0), stop=(k == KT - 1),
                    )
                    if warm and k < 7:
                        warm_mm()
                return pa, pb

            def qkv_post(j, m, ps):
                """rope for head-pair m of chunk j: copy psum->sbuf f32r,
                perm matmul, then q*cos + perm(q)*sin on DVE."""
                xs = slice(j * LQB, (j + 1) * LQB)
                raw = rawpool.tile([128, LQB], F32R, tag="raw", name="raw")
                nc.vector.tensor_copy(raw[:], ps[:])
                psw = qpool.tile([128, 512], F32, tag="q", name=f"psw{j}_{m}")
                nc.tensor.matmul(psw[:], perm_sb[:], raw[:],
                                 start=True, stop=True)
                t1 = tpool.tile([128, LQB], F32, tag="t1", name="t1")
                nc.vector.tensor_mul(t1[:], raw[:].bitcast(F32), cos_sb[:, xs])
                t2 = tpool.tile([128, LQB], F32, tag="t2", name="t2")
                nc.vector.tensor_mul(t2[:], psw[:], sin_sb[:, xs])
                nc.vector.tensor_add(qk_rot[m][:, xs], t1[:], t2[:])

            def v_unit(j, i2):
                """one L-tile of V for chunk j: matmul chain + copy."""
                ti = j * 4 + i2
                psv = qpool.tile([128, 512], F32, tag="q", name=f"psv{ti}")
                for k in range(KT):
                    nc.tensor.matmul(
                        psv[:, 0:260], xt_t[(j, k)][:, i2 * 128:(i2 + 1) * 128],
                        wvt_sb[k],
                        start=(k == 0), stop=(k == KT - 1),
                    )
                if ti % 2 == 0:
                    nc.vector.tensor_copy(v_sb[ti][:], psv[:, 0:260])
                else:
                    nc.scalar.copy(v_sb[ti][:], psv[:, 0:260])
                nc.gpsimd.memset(v_sb[ti][:, 64:260:65], 1.0)

            # attention tile units: se = scores+exp(+mask), pv = P@V
            pt_of = {}

            def att_se(jq, hp, t):
                nt = 4 * jq + 4
                diag = t >= 4 * jq
                off = max(0, t * 128 - jq * LQB)
                off_mm = min(off, 256)   # f32r needs >=256 moving rows
                r = off // 128
                ks = slice(t * 128, (t + 1) * 128)
                sp = sppool.tile([128, 1024], F32, tag="sp", name="sp")
                for h in range(2):
                    hs = slice(64 * h, 64 * h + 64)
                    nc.tensor.matmul(
                        sp[:, 512 * h + off_mm:512 * h + 512],
                        qk_rot[2 + hp][hs, ks],
                        qk_rot[hp][hs, jq * LQB + off_mm:(jq + 1) * LQB],
                        start=True, stop=not (diag and o["pe_mask"]),
                    )
                if diag and o["pe_mask"]:
                    for h in range(2):
                        nc.tensor.matmul(
                            sp[:, 512 * h + off:512 * h + off + 128],
                            negi_sb[:], bigm_sb[:],
                            start=False, stop=True,
                        )
                pt = ptpool.tile([128, 1024], F16, tag="pt", name="pt")
                if off >= 256:
                    for h in range(2):
                        nc.scalar.activation(
                            pt[:, 512 * h + off:512 * h + 512],
                            sp[:, 512 * h + off:512 * h + 512],
                            mybir.ActivationFunctionType.Exp)
                else:
                    nc.scalar.activation(
                        pt[:, off:1024], sp[:, off:1024],
                        mybir.ActivationFunctionType.Exp)
                if diag and not o["pe_mask"]:
                    # zero exp(S) above the diagonal in the diag 128-block
                    for h in range(2):
                        c0 = 512 * h + 128 * r
                        nc.vector.tensor_mul(pt[:, c0:c0 + 128],
                                             pt[:, c0:c0 + 128], tri[:])
                pt_of[(jq, hp, t)] = pt

            def att_pv(jq, hp, t):
                nt = 4 * jq + 4
                off = max(0, t * 128 - jq * LQB)
                r = off // 128
                if t == 0:
                    # zero via DVE, then accumulate with start=False
                    # throughout: matmul start=True zeroes the WHOLE psum
                    # bank, which clobbers sibling js-regions when Tile
                    # reorders the (commutative) accumulate matmuls.  Emitted
                    # here (not in att_se) so the DVE stream keeps it after
                    # the previous pair's normalize.
                    for h in range(2):
                        nc.vector.memset(y_ps[h][:], 0.0)
                pt = pt_of.pop((jq, hp, t))
                for h in range(2):
                    H = 2 * hp + h
                    for js in range(r, 4):
                        nc.tensor.matmul(
                            y_ps[h][:, 65 * js:65 * js + 65],
                            pt[:, 512 * h + 128 * js:512 * h + 128 * js + 128],
                            v_sb[t][:, 65 * H:65 * H + 65],
                            start=False, stop=(t == nt - 1),
                            skip_group_check=True,
                        )
                if t == nt - 1:
                    for h in range(2):
                        H = 2 * hp + h
                        rec = recpool.tile([128, 4], F32, tag="rec", name="rec")
                        nc.vector.reciprocal(rec[:], y_ps[h][:, 64:260:65])
                        for js in range(4):
                            i = 4 * jq + js
                            nc.vector.tensor_scalar_mul(
                                y_all[i][:, HD * H:HD * H + HD],
                                y_ps[h][:, 65 * js:65 * js + 64],
                                rec[:, js:js + 1],
                            )

            def wo_unit(i, fine=False):
                """project one finished L-tile through wo and stream out.
                fine=True (tail tiles): split copies over both engines and
                the OUT DMA over 4 queues/triggers to shrink the tail."""
                tp = qpool.tile([128, 512], F32, tag="q", name=f"tp{i}")
                tp16 = tp[:].bitcast(F16)
                for c2 in range(2):
                    nc.tensor.transpose(
                        tp16[:, 128 * c2:128 * c2 + 128],
                        y_all[i][:, 128 * c2:128 * c2 + 128],
                        idn[:],
                    )
                    nc.vector.tensor_copy(
                        yt_sb[c2][:, 128 * i:128 * i + 128],
                        tp16[:, 128 * c2:128 * c2 + 128],
                    )
                ob = opool.tile([128, 1024], F16, tag="ob", name="ob")
                out_engs = [nc.gpsimd, nc.scalar, nc.sync, nc.gpsimd]
                for half in range(2):
                    po = qpool.tile([128, 512], F32, tag="q", name=f"po{i}")
                    for c2 in range(2):
                        nc.tensor.matmul(
                            po[:],
                            yt_sb[c2][:, 128 * i:128 * i + 128],
                            wot_sb[c2][:, 512 * half:512 * half + 512],
                            start=(c2 == 0), stop=(c2 == 1),
                        )
                    hs = slice(512 * half, 512 * half + 512)
                    if fine:
                        for quad in range(2):
                            qs = slice(512 * half + 256 * quad,
                                       512 * half + 256 * quad + 256)
                            ps = slice(256 * quad, 256 * quad + 256)
                            if quad == 0:
                                nc.scalar.copy(ob[:, qs], po[:, ps])
                            else:
                                nc.vector.tensor_copy(ob[:, qs], po[:, ps])
                            out_engs[2 * half + quad].dma_start(
                                OUT[128 * i:128 * i + 128, qs], ob[:, qs])
                    else:
                        if half == 0:
                            nc.scalar.copy(ob[:, hs], po[:])
                        else:
                            nc.vector.tensor_copy(ob[:, hs], po[:])
                        nc.gpsimd.dma_start(OUT[128 * i:128 * i + 128, hs],
                                            ob[:, hs])

            # ---- emission: fused global schedule ------------------------
            # qkv chunk 0 (DMA-paced: interleave the two hp0 chains k-by-k
            # and sprinkle warmup matmuls)
            pa, pb = qkv_chain2(0, 0, 2, warm=True)
            qkv_post(0, 0, pa)
            qkv_post(0, 2, pb)
            pa, pb = qkv_chain2(0, 1, 3)
            qkv_post(0, 1, pa)
            qkv_post(0, 3, pb)

            for jq in range(NLQ):
                nt = 4 * jq + 4
                # fill units: qkv of chunk jq+1, v of chunk jq (needed by
                # this chunk's diag tiles) emitted first, wo of chunk jq-1
                fills = []
                for i2 in range(4):
                    fills.append(lambda j=jq, i2=i2: v_unit(j, i2))
                if jq + 1 < NLQ:
                    def mk_chain(m):
                        def f(j=jq + 1, m=m):
                            ps = qkv_chain(j, m)
                            qkv_post(j, m, ps)
                        return f
                    for m in (0, 2, 1, 3):
                        fills.append(mk_chain(m))
                if jq > 0:
                    for i in range(4 * (jq - 1), 4 * (jq - 1) + 4):
                        fills.append(lambda i=i: wo_unit(i))

                units = [(hp, t) for hp in range(2) for t in range(nt)]
                LOOK = o["look"]
                nf = len(fills)
                nu = len(units)
                fi = 0

                def need_fills(hp, t):
                    # v units (fills[0..3]) must be emitted before the pv
                    # (and se, via lookahead) units that consume v_sb of
                    # this chunk's diagonal tiles: a consumer emitted before
                    # its producer reads stale SBUF (Tile orders by
                    # emission, it does not reorder to satisfy RAW).
                    d = t - 4 * jq
                    return min(4, d + 1) if d >= 0 else 0

                for ui, (hp, t) in enumerate(units):
                    if ui == 0:
                        for la in range(min(LOOK, nu)):
                            att_se(jq, *units[la])
                    elif ui + LOOK - 1 < nu:
                        att_se(jq, *units[ui + LOOK - 1])
                    # spread fills across the pv walk
                    while fi < max(nf * (ui + 1) // nu, need_fills(hp, t)):
                        fills[fi]()
                        fi += 1
                    att_pv(jq, hp, t)
                while fi < nf:
                    fills[fi]()
                    fi += 1

            for i in range(4 * (NLQ - 1), 4 * NLQ):
                wo_unit(i, fine=(i >= 4 * NLQ - 2))

    nc.finalize()
    return nc


def prep_inputs(x, wqkv, wo):
    """Build the 8 per-core input dicts from the full-problem inputs."""
    x = np.asarray(x, dtype=np.float32)
    wqkv = np.asarray(wqkv, dtype=np.float32)
    wo = np.asarray(wo, dtype=np.float32)

    # rope tables
    inv_freq = 1.0 / (10000.0 ** (np.arange(0, HD, 2, dtype=np.float32) / HD))
    t = np.arange(L, dtype=np.float32)
    freqs = np.outer(t, inv_freq)                  # (L, 32)
    cos32 = np.cos(freqs).T.astype(np.float32)     # (32, L)
    sin32 = np.sin(freqs).T.astype(np.float32)
    COS = np.ascontiguousarray(np.tile(cos32, (4, 1)))           # (128, L)
    SIN = np.ascontiguousarray(
        np.concatenate([-sin32, sin32, -sin32, sin32], axis=0)
    )

    # 32-block swap permutation (within each head's 64 rows)
    PERM = np.zeros((128, 128), dtype=np.float32)
    for blk in range(2):
        o = 64 * blk
        PERM[o:o + 32, o + 32:o + 64] = np.eye(32)
        PERM[o + 32:o + 64, o:o + 32] = np.eye(32)

    in_maps = []
    scale = np.float32(HD ** -0.5)
    for c in range(NCORES):
        b, g = divmod(c, 4)
        qrows = slice(256 * g, 256 * g + 256)
        krows = slice(1024 + 256 * g, 1024 + 256 * g + 256)
        vrows = slice(2048 + 256 * g, 2048 + 256 * g + 256)

        XT = np.ascontiguousarray(x[b].T)                        # (1024, 2048)
        wq = (wqkv[qrows, :] * scale).T                          # (1024, 256)
        wk = wqkv[krows, :].T
        WQKT = np.ascontiguousarray(np.concatenate([wq, wk], axis=1))
        vpart = wqkv[vrows, :].T                                 # (1024, 256)
        WVT0 = np.zeros((D, 260), dtype=np.float32)
        for h in range(HPC):
            WVT0[:, 65 * h:65 * h + 64] = vpart[:, 64 * h:64 * h + 64]
        # pack k-tiles side by side: [128, 8*260] so two fat DMAs load all
        WVT = np.concatenate([WVT0[k * 128:(k + 1) * 128, :]
                              for k in range(KT)], axis=1)
        WVT = np.ascontiguousarray(WVT)
        WOT = np.ascontiguousarray(wo[:, 256 * g:256 * g + 256].T)

        in_maps.append({
            "XT": XT.astype(np.float16),
            "WQKT": WQKT.astype(np.float16),
            "WVT": WVT.astype(np.float16),
            "WOT": WOT.astype(np.float16),
            "COS": COS.astype(np.float16),
            "SIN": SIN.astype(np.float16),
            "PERM": PERM,
        })
    return in_maps


def kernel(x, wqkv, wo):
    if "nc" not in _cache:
        _cache["nc"] = build_nc()
    nc = _cache["nc"]
    in_maps = prep_inputs(x, wqkv, wo)
    res = bass_utils.run_bass_kernel_spmd(nc, in_maps, list(range(NCORES)))
    outs = [res.results[c]["OUT"].astype(np.float32) for c in range(NCORES)]
    out0 = outs[0] + outs[1] + outs[2] + outs[3]
    out1 = outs[4] + outs[5] + outs[6] + outs[7]
    return np.stack([out0, out1]).astype(np.float32)
